# revision 39
# baseline (speedup 1.0000x reference)
"""Multi-head attention (RoPE) Trainium2 Bass kernel.

Problem: B=2, S=2048, d_model=1024, 16 heads x head_dim 64, fp32.

The reference faithfully replicates a torch rank-5 reshape bug: the
attention output [1,H,B,S,D] is transposed to [1,H,B,S,D]->(0,2,1,3,4)
and flat-reshaped to [B,S,H*D] BEFORE the Wo projection. Net semantics:
  out[b2, s2, :] = flatten(O[b, h, s0:s0+16, :]) @ Wo + bo
  with h = b2*8 + s2//256, b = (s2//128)%2, s0 = (s2%128)*16,
so the projection is PER-HEAD (contraction mixes 16 seq x 64 dims of one
head) and every (b,h) yields an independent [128, 1024] output block.

Sharding (8 cores): batch (2) x head groups (4 groups of 4 heads).

Schedule: the softmax exp is the hard floor -- 4 heads x S^2 = 16.8M
exps per core on the ACT engine (~140 us of 1.2 GHz spline evals).
Everything else hides under it:
  * prologue: Q, K, V projections for ALL seq tiles (the prologue is
    DMA-paced, so the extra Q matmuls ride free on idle PE slack).
    RoPE needs no matmul at all: rotate_half is a 32-partition block
    swap, done by one SBUF->SBUF DMA, with the sign pattern baked into
    a host-side signed-sin table.
  * stream over 4 q-tiles x 2 head-pairs: scores -> exp -> AV per
    128-key chunk, triple-buffered score PSUM, AV lagging LAG chunks.
    Softmax denominators: ones-column in V gives sums; one batched
    [2,512] reciprocal_approx_fast per head-pair, DRAM-bounce partition
    broadcast, normalize muls write the Wo-packed layout directly.
  * tail: per-head scrambled Wo projection with j-PAIRS packed across
    the partition dim (K=128 per matmul, 8 matmuls per output chunk).
The PE's HAM clock gate starts at 1.2 GHz and re-throttles after idle
gaps; junk warm-up matmuls at kernel start and across the normalize
tail keep it at 2.4 GHz.
"""

import numpy as np

import concourse.bass as bass
import concourse.tile as tile
from concourse import bacc, mybir
from concourse import bass_utils

F32 = mybir.dt.float32
MM_DT = mybir.dt.float32r  # matmul operand dtype (float32r: 1 cyc/row)

B, S, DM, H, HD = 2, 2048, 1024, 16, 64
N_CORES = 8
HG = 4          # head groups (tensor-parallel factor)
GD = DM // HG   # qkv dims per core = 256
NKC = DM // 128   # d_model contraction chunks = 8
NST = S // 512    # seq tiles of 512 = 4
NSK = S // 128    # seq_k chunks of 128 = 16
LAG = 2           # AV matmuls trail score matmuls by LAG kc iterations


def _emit(nc, tc, ap, debug=False):
    import contextlib

    ctx = contextlib.ExitStack()
    with ctx:
        consts = ctx.enter_context(tc.tile_pool(name="consts", bufs=1))
        big = ctx.enter_context(tc.tile_pool(name="big", bufs=1))

        # ---- PE warm-up: the HAM clock gate keeps an idle PE at 1.2 GHz and
        # re-warms only after ~3.4us of sustained matmul activity. Spin junk
        # matmuls while the weight DMAs are in flight so real work runs at
        # 2.4 GHz from the start.
        warm_sb = consts.tile([128, 512], MM_DT, name="warm_sb")
        nc.vector.memset(warm_sb.bitcast(F32), 0.0)
        with tc.tile_pool(name="ps_w", bufs=1, space="PSUM") as ps_w:
            warm_ps = ps_w.tile([128, 512], F32, name="warm_ps")
            for _ in range(14):
                nc.tensor.matmul(
                    warm_ps, lhsT=warm_sb[:, 0:128], rhs=warm_sb,
                    start=True, stop=True,
                )

        # ---- constants / weights to SBUF ----
        # per-kc chunks, kc-major round-robin across the three DMA trigger
        # queues: each queue sustains ~1 transfer per ~1.5us, so kc=0's full
        # working set (wk/wq/wv/xt) must sit at the FRONT of every queue.
        xt_pool = ctx.enter_context(tc.tile_pool(name="xt", bufs=8))
        wk_c, wq_c, wv_c, xq0 = [], [], [], []
        qs = [nc.sync, nc.scalar, nc.gpsimd]
        qi = 0
        for kc in range(NKC):
            ksl = slice(kc * 128, (kc + 1) * 128)
            t = consts.tile([128, GD], MM_DT, name=f"wk{kc}", tag=f"wk{kc}")
            qs[qi % 3].dma_start(t, ap["wk"][ksl, :].bitcast(MM_DT)); qi += 1
            wk_c.append(t)
            t = consts.tile([128, GD], MM_DT, name=f"wq{kc}", tag=f"wq{kc}")
            qs[qi % 3].dma_start(t, ap["wq"][ksl, :].bitcast(MM_DT)); qi += 1
            wq_c.append(t)
            t = consts.tile([128, GD], MM_DT, name=f"wv{kc}", tag=f"wv{kc}")
            qs[qi % 3].dma_start(t, ap["wv"][ksl, :].bitcast(MM_DT)); qi += 1
            wv_c.append(t)
            t = xt_pool.tile([128, 512], MM_DT, name="xtp", tag="xtp")
            qs[qi % 3].dma_start(
                t, ap["xt"][ksl, 0:512].bitcast(MM_DT)
            ); qi += 1
            xq0.append(t)
        cosb = consts.tile([128, S], F32)
        nc.sync.dma_start(cosb, ap["cosb"])
        # sign-baked sin table: ssin rows 0-31 of each 64-block carry -sin,
        # rows 32-63 carry +sin -- so rotate_half reduces to a partition
        # block-swap plus an elementwise multiply
        ssinb = consts.tile([128, S], F32)
        nc.scalar.dma_start(ssinb, ap["ssinb"])
        bqc = consts.tile([128, 2], F32)
        nc.gpsimd.dma_start(bqc, ap["bq2"].rearrange("c p -> p c"))
        bkc = consts.tile([128, 2], F32)
        nc.gpsimd.dma_start(bkc, ap["bk2"].rearrange("c p -> p c"))
        bvb = consts.tile([128, GD], F32)
        nc.gpsimd.dma_start(bvb, ap["bv"].partition_broadcast(128))

        # ---- persistent activation buffers ----
        # qe split per q-tile (separate tiles avoid whole-tile false deps)
        qet = [
            [
                big.tile([128, 512], MM_DT, name=f"qe{qt}{mc}", tag=f"qe{qt}{mc}")
                for mc in range(2)
            ]
            for qt in range(NST)
        ]
        ket = [
            [
                big.tile([128, 512], MM_DT, name=f"ke{st}{mc}", tag=f"ke{st}{mc}")
                for mc in range(2)
            ]
            for st in range(NST)
        ]
        # V natural layout + ones column: [128 seq, kc, head, 65]
        vsb = big.tile([128, NSK, 4, 65], MM_DT, name="vsb", tag="vsb")
        nc.vector.memset(vsb[:, :, :, 64:65].bitcast(F32), 1.0)
        # normalized attention output, packed for the Wo projection:
        # ot2[p, h, s2r, pj] with partitions 0-63 = O^T[d, q=s2r*16+2pj]
        # (even j) and 64-127 = O^T[d, q=s2r*16+2pj+1] (odd j), so each Wo
        # matmul contracts a j-PAIR with K=128.
        ot2 = big.tile([128, 4, 128, 8], MM_DT, name="ot2", tag="ot2")

        def emit_rope(rr_pool, t1_pool, raws, dsts, sl, eng=None, mul_eng=None):
            # dsts[mc] = raw*cos + blockswap32(raw)*ssin  (no PE, no PSUM)
            eng = eng or nc.gpsimd
            mul_eng = mul_eng or nc.vector
            for mc in range(2):
                rr = rr_pool.tile([128, 512], MM_DT, name="rr", tag="rr")
                for g in range(0, 128, 64):
                    eng.dma_start(
                        rr[g:g + 32, :], raws[mc][g + 32:g + 64, :]
                    )
                    eng.dma_start(
                        rr[g + 32:g + 64, :], raws[mc][g:g + 32, :]
                    )
                t1 = t1_pool.tile([128, 512], F32, name="t1", tag="t1")
                mul_eng.tensor_mul(t1, rr, ssinb[:, sl])
                d = dsts[mc]
                nc.vector.tensor_mul(d, raws[mc], cosb[:, sl])
                nc.vector.tensor_add(d, d, t1)

        # ====== Prologue: Q, K, V projections for ALL seq tiles ======
        # DMA-paced (8MB of x^T), so the Q matmuls ride on idle PE slack.
        with (
            tc.tile_pool(name="raw", bufs=2) as raw_pool,
            tc.tile_pool(name="rr", bufs=4) as rr_pool,
            tc.tile_pool(name="t1", bufs=2) as t1_pool,
            tc.tile_pool(name="ps_k", bufs=2, space="PSUM") as ps_k,
            tc.tile_pool(name="ps_q", bufs=2, space="PSUM") as ps_q,
            tc.tile_pool(name="ps_v", bufs=4, space="PSUM") as ps_v,
        ):
            for st in range(NST):
                sl = slice(st * 512, (st + 1) * 512)
                pk = {}
                pq = {}
                for mc in range(2):
                    pk[mc] = ps_k.tile([128, 512], F32, name=f"pk{mc}", tag="pk")
                    pq[mc] = ps_q.tile([128, 512], F32, name=f"pq{mc}", tag="pq")
                pv = {}
                for ss in range(4):
                    pv[ss] = ps_v.tile([128, GD], F32, name=f"pv{ss}", tag="pv")
                for kc in range(NKC):
                    if st == 0:
                        xt_kc = xq0[kc]
                    else:
                        xt_kc = xt_pool.tile([128, 512], MM_DT, name="xtp", tag="xtp")
                        eng = qs[(st * NKC + kc) % 3]
                        eng.dma_start(
                            xt_kc,
                            ap["xt"][kc * 128:(kc + 1) * 128, sl].bitcast(MM_DT),
                        )
                    for mc in range(2):
                        nc.tensor.matmul(
                            pk[mc],
                            lhsT=wk_c[kc][:, mc * 128:(mc + 1) * 128],
                            rhs=xt_kc,
                            start=(kc == 0),
                            stop=(kc == NKC - 1),
                        )
                    for mc in range(2):
                        nc.tensor.matmul(
                            pq[mc],
                            lhsT=wq_c[kc][:, mc * 128:(mc + 1) * 128],
                            rhs=xt_kc,
                            start=(kc == 0),
                            stop=(kc == NKC - 1),
                        )
                    for ss in range(4):
                        nc.tensor.matmul(
                            pv[ss],
                            lhsT=xt_kc[:, ss * 128:(ss + 1) * 128],
                            rhs=wv_c[kc],
                            start=(kc == 0),
                            stop=(kc == NKC - 1),
                        )
                # drains (all DVE/DMA -- the PE rolls straight into st+1)
                raws = {}
                rawsq = {}
                for mc in range(2):
                    raw = raw_pool.tile([128, 512], MM_DT, name=f"rawk{mc}", tag=f"rawk{mc}")
                    nc.vector.tensor_scalar_add(raw, pk[mc], bkc[:, mc:mc + 1])
                    raws[mc] = raw
                    raw = raw_pool.tile([128, 512], MM_DT, name=f"rawq{mc}", tag=f"rawq{mc}")
                    nc.vector.tensor_scalar_add(raw, pq[mc], bqc[:, mc:mc + 1])
                    rawsq[mc] = raw
                for ss in range(4):
                    # stays on DVE: gpsimd cannot read PSUM (P2)
                    nc.vector.tensor_add(
                        vsb[:, st * 4 + ss, :, 0:64],
                        pv[ss].rearrange("p (h d) -> p h d", h=4),
                        bvb.rearrange("p (h d) -> p h d", h=4),
                    )
                emit_rope(rr_pool, t1_pool, raws, [ket[st][0], ket[st][1]], sl)
                emit_rope(rr_pool, t1_pool, rawsq, [qet[st][0], qet[st][1]], sl,
                          eng=nc.scalar, mul_eng=nc.gpsimd)

        wo_pool = ctx.enter_context(tc.tile_pool(name="wo_mc", bufs=4))
        wo_tiles = {}

        # ================= Stream: scores -> exp -> AV =================
        with (
            tc.tile_pool(name="e", bufs=LAG + 3) as e_pool,
            tc.tile_pool(name="usb", bufs=5) as usb_pool,
            tc.tile_pool(name="dens", bufs=2) as dens_pool,
            tc.tile_pool(name="dbc", bufs=4) as dbc_pool,
            tc.tile_pool(name="todd", bufs=4) as todd_pool,
            tc.tile_pool(name="rdram", bufs=2, space="DRAM") as rdram_pool,
            tc.tile_pool(name="ps_s", bufs=3, space="PSUM") as ps_s,
            tc.tile_pool(name="ps_u", bufs=1, space="PSUM") as ps_u,
        ):
            for qt in range(NST):
                if qt == 1:
                    # prefetch half of Wo now: sync is idle mid-stream
                    # (at prologue time these transfers starve the xt loads)
                    for mc in range(4):
                        wo_mc = wo_pool.tile(
                            [128, 8, 128], MM_DT, name="wo_mc", tag="wo_mc"
                        )
                        nc.sync.dma_start(
                            wo_mc,
                            ap["wo"][:, mc * 128:(mc + 1) * 128]
                            .rearrange("(c p) m -> p c m", p=128)
                            .bitcast(MM_DT),
                        )
                        wo_tiles[mc] = wo_mc
                for hc in range(2):
                    u = [
                        ps_u.tile([65, 512], F32, name=f"u{i}", tag=f"u{i}")
                        for i in range(2)
                    ]
                    es = {}
                    for kc in range(NSK + LAG):
                        if kc >= LAG:
                            ka = kc - LAG
                            for hi in range(2):
                                nc.tensor.matmul(
                                    u[hi],
                                    lhsT=vsb[:, ka, hc * 2 + hi, :],
                                    rhs=es[ka][:, hi * 512:(hi + 1) * 512],
                                    start=(ka == 0),
                                    stop=(ka == NSK - 1),
                                )
                            if ka > 0:
                                del es[ka - 1]
                        if kc < NSK:
                            # both heads' scores side by side in one 2-bank
                            # group
                            g = ps_s.tile([128, 1024], F32, tag="sg", name="sg")
                            for hi in range(2):
                                hpart = slice(hi * 64, (hi + 1) * 64)
                                nc.tensor.matmul(
                                    g[:, hi * 512:(hi + 1) * 512],
                                    lhsT=ket[kc // 4][hc][
                                        hpart, (kc % 4) * 128:(kc % 4 + 1) * 128
                                    ],
                                    rhs=qet[qt][hc][hpart, :],
                                    start=True,
                                    stop=True,
                                )
                            e = e_pool.tile([128, 1024], MM_DT, name="e", tag="e")
                            nc.scalar.activation(
                                e, g, mybir.ActivationFunctionType.Exp, scale=0.125
                            )
                            es[kc] = e
                    # ---- drain + normalize this head-pair ----
                    densh = dens_pool.tile([2, 512], MM_DT, name="dens", tag="dens")
                    usbs = {}
                    for hi in range(2):
                        # copy U off PSUM immediately so the bank frees early
                        usb = usb_pool.tile([65, 512], MM_DT, name="usb", tag="usb")
                        nc.vector.tensor_copy(usb, u[hi])
                        usbs[hi] = usb
                        # denominator row gather by DMA (engine ops can't
                        # write at partition base 1)
                        nc.gpsimd.dma_start(densh[hi:hi + 1, :], usb[64:65, :])
                    if qt == NST - 1 and hc == 1:
                        # junk matmuls bridging the AV-end -> usb-copy window
                        wps = ps_s.tile([128, 512], F32, name="wps", tag="sg")
                        for _ in range(6):
                            nc.tensor.matmul(
                                wps, lhsT=warm_sb[:, 0:128], rhs=warm_sb,
                                start=True, stop=True,
                            )
                        for _ in range(14):
                            nc.tensor.matmul(
                                wps, lhsT=warm_sb[0:65, 0:128],
                                rhs=usbs[1], start=True, stop=True,
                            )
                    rcp = dens_pool.tile([2, 512], F32, name="rcp", tag="rcp")
                    nc.vector.reciprocal_approx_fast(out=rcp, in_=densh.bitcast(F32))
                    rd = rdram_pool.tile([2, 512], MM_DT, name="rd", tag="rd")
                    nc.sync.dma_start(rd, rcp.bitcast(MM_DT))
                    for hi in range(2):
                        hh = hc * 2 + hi
                        dbc_sb = dbc_pool.tile([64, 512], MM_DT, name="dbc", tag="dbc")
                        nc.sync.dma_start(
                            dbc_sb, rd[hi:hi + 1, :].partition_broadcast(64)
                        )
                        if qt == NST - 1 and hc == 1:
                            # bridge matmuls gated on the broadcast tile
                            wps3 = ps_s.tile(
                                [128, 512], F32, name="wps3", tag="sg"
                            )
                            for _ in range(3):
                                nc.tensor.matmul(
                                    wps3, lhsT=warm_sb[0:64, 0:128],
                                    rhs=dbc_sb, start=True, stop=True,
                                )
                        usb_v = usbs[hi][0:64, :].rearrange(
                            "p (a b c) -> p a b c", b=8, c=2
                        )
                        dbc_v = dbc_sb.rearrange("p (a b c) -> p a b c", b=8, c=2)
                        # even j: strided write straight into ot2 parts 0-63
                        nc.vector.tensor_mul(
                            ot2[0:64, hh, qt * 32:(qt + 1) * 32, :],
                            usb_v[:, :, :, 0], dbc_v[:, :, :, 0],
                        )
                        # odd j: DVE lanes can't cross partitions, so stage
                        # contiguously and DMA into partitions 64-127
                        todd = todd_pool.tile(
                            [64, 32, 8], MM_DT, name="todd", tag="todd"
                        )
                        nc.vector.tensor_mul(
                            todd, usb_v[:, :, :, 1], dbc_v[:, :, :, 1]
                        )
                        nc.gpsimd.dma_start(
                            ot2[64:128, hh, qt * 32:(qt + 1) * 32, :], todd
                        )
                        if qt == NST - 1 and hc == 1:
                            # more bridge matmuls gated on this head's todd
                            # tile -- they run while the final DMAs land,
                            # keeping the clock warm into the projection
                            wps2 = ps_s.tile(
                                [128, 512], F32, name="wps2", tag="sg"
                            )
                            for _ in range(4):
                                nc.tensor.matmul(
                                    wps2[:, 0:256], lhsT=warm_sb[0:64, 0:128],
                                    rhs=todd, start=True, stop=True,
                                )

        # ====== Tail: per-head scrambled output projection (full Wo) ======
        # out block rows mix 16 seq x 64 dim: row s2r contracts
        # O[s2r*16 + j, d] * Wo[j*64 + d, :].  ot2 packs j-pairs across the
        # partition dim, so each matmul contracts K=128 (j even at 0-63, odd
        # at 64-127) -- 8 matmuls per mc instead of 16.
        with (
            tc.tile_pool(name="ysb", bufs=3) as y_pool,
            tc.tile_pool(name="ps_y", bufs=3, space="PSUM") as ps_y,
        ):
            for mc in range(NKC):
                if mc in wo_tiles:
                    wo_mc = wo_tiles[mc]
                else:
                    wo_mc = wo_pool.tile([128, 8, 128], MM_DT, name="wo_mc", tag="wo_mc")
                    nc.gpsimd.dma_start(
                        wo_mc,
                        ap["wo"][:, mc * 128:(mc + 1) * 128]
                        .rearrange("(c p) m -> p c m", p=128)
                        .bitcast(MM_DT),
                    )
                py = ps_y.tile([128, 512], F32, name="py", tag="py")
                for pj in range(8):
                    nc.tensor.matmul(
                        py,
                        lhsT=wo_mc[:, pj, :],
                        rhs=ot2[:, :, :, pj],
                        start=(pj == 0),
                        stop=(pj == 7),
                    )
                ysb = y_pool.tile([128, 512], F32, name="ysb", tag="ysb")
                nc.vector.tensor_copy(ysb, py)
                nc.sync.dma_start(ap["ypt"][mc * 128:(mc + 1) * 128, :], ysb)


def _build(debug=False):
    nc = bacc.Bacc("TRN2", target_bir_lowering=False, debug=False, num_devices=N_CORES)
    ap = {}
    ap["xt"] = nc.dram_tensor("xt", [DM, S], F32, kind="ExternalInput").ap()
    ap["wq"] = nc.dram_tensor("wq", [DM, GD], F32, kind="ExternalInput").ap()
    ap["wk"] = nc.dram_tensor("wk", [DM, GD], F32, kind="ExternalInput").ap()
    ap["wv"] = nc.dram_tensor("wv", [DM, GD], F32, kind="ExternalInput").ap()
    ap["wo"] = nc.dram_tensor("wo", [DM, DM], F32, kind="ExternalInput").ap()
    ap["bq2"] = nc.dram_tensor("bq2", [2, 128], F32, kind="ExternalInput").ap()
    ap["bk2"] = nc.dram_tensor("bk2", [2, 128], F32, kind="ExternalInput").ap()
    ap["bv"] = nc.dram_tensor("bv", [GD], F32, kind="ExternalInput").ap()
    ap["cosb"] = nc.dram_tensor("cosb", [128, S], F32, kind="ExternalInput").ap()
    ap["ssinb"] = nc.dram_tensor("ssinb", [128, S], F32, kind="ExternalInput").ap()
    # per-core output: Y^T [1024, 512] (columns = 4 heads x 128 block rows)
    ap["ypt"] = nc.dram_tensor("ypt", [DM, 512], F32, kind="ExternalOutput").ap()
    if debug:
        ap["ke_dbg"] = nc.dram_tensor("ke_dbg", [2, 128, S], F32, kind="ExternalOutput").ap()
        ap["v_dbg"] = nc.dram_tensor("v_dbg", [128, NSK, 4, 65], F32, kind="ExternalOutput").ap()

    with tile.TileContext(nc) as tc:
        _emit(nc, tc, ap, debug=debug)
    nc.compile()
    return nc


_CACHE = {}


def _rope_tables():
    inv_freq = (1.0 / (10000.0 ** (np.arange(0, HD, 2, dtype=np.float32) / HD))).astype(np.float32)
    t = np.arange(S, dtype=np.float32)
    freqs = np.outer(t, inv_freq).astype(np.float32)  # [S, 32]
    emb = np.concatenate([freqs, freqs], axis=-1)  # [S, 64]
    cosT = np.cos(emb).astype(np.float32).T  # [64, S]
    sinT = np.sin(emb).astype(np.float32).T
    cosb = np.ascontiguousarray(np.concatenate([cosT, cosT], axis=0))  # [128, S]
    # sign-baked sin: rotate_half contributes -x[d+32]*sin[d] for d<32 and
    # +x[d-32]*sin[d] for d>=32; after the partition block-swap the sign is
    # a pure function of the destination row
    sgn = np.ones((64, 1), np.float32)
    sgn[0:32] = -1.0
    ssinT = sinT * sgn
    ssinb = np.ascontiguousarray(np.concatenate([ssinT, ssinT], axis=0))
    return cosb, ssinb


def kernel(x, Wq, bq, Wk, bk, Wv, bv, Wo, bo):
    x = np.asarray(x, dtype=np.float32)
    Wq, bq = np.asarray(Wq, np.float32), np.asarray(bq, np.float32)
    Wk, bk = np.asarray(Wk, np.float32), np.asarray(bk, np.float32)
    Wv, bv = np.asarray(Wv, np.float32), np.asarray(bv, np.float32)
    Wo, bo = np.asarray(Wo, np.float32), np.asarray(bo, np.float32)

    if "nc" not in _CACHE:
        _CACHE["nc"] = _build()
    nc = _CACHE["nc"]

    cosb, ssinb = _rope_tables()
    xt_b = [np.ascontiguousarray(x[b].T) for b in range(B)]  # [DM, S]
    wo_c = np.ascontiguousarray(Wo)

    in_maps = []
    for c in range(N_CORES):
        b, hg = divmod(c, HG)
        sl = slice(hg * GD, (hg + 1) * GD)
        in_maps.append(
            {
                "xt": xt_b[b],
                "wq": np.ascontiguousarray(Wq[:, sl]),
                "wk": np.ascontiguousarray(Wk[:, sl]),
                "wv": np.ascontiguousarray(Wv[:, sl]),
                "wo": wo_c,
                "bq2": np.ascontiguousarray(bq[sl].reshape(2, 128)),
                "bk2": np.ascontiguousarray(bk[sl].reshape(2, 128)),
                "bv": np.ascontiguousarray(bv[sl]),
                "cosb": cosb,
                "ssinb": ssinb,
            }
        )

    res = bass_utils.run_bass_kernel_spmd(nc, in_maps, core_ids=list(range(N_CORES)))
    _CACHE["last_results"] = res

    # Block placement: core (b, hg), local head hl -> global head h = hg*4+hl,
    # lands at out[h//8, (h%8)*256 + b*128 : +128, :].
    out = np.empty((B, S, DM), dtype=np.float32)
    for c in range(N_CORES):
        b, hg = divmod(c, HG)
        ypt = res.results[c]["ypt"]  # [1024, 512]
        for hl in range(4):
            h = hg * 4 + hl
            b2 = h // 8
            s2 = (h % 8) * 256 + b * 128
            out[b2, s2:s2 + 128, :] = ypt[:, hl * 128:(hl + 1) * 128].T
    out += bo[None, None, :]
    return out
